# revision 7
# baseline (speedup 1.0000x reference)
"""AGNNConv on 8 Trainium2 NeuronCores — dense matmul formulation.

The per-edge attention weight exp(beta * cos(src, dst)) depends only on the
(src, dst) node pair, so the whole message passing collapses to dense algebra:

    G = norm^T norm                  (Gram matrix of L2-normalized features)
    H = C  *  exp(beta * G)          (C = dense dst-by-src edge-count matrix)
    num|den = H^T @ [feat | 1]  ;    out = num / den  rowwise

The count matrix C (dense, from the edge list), the L2-normalized transposed
features, and the [feat | 1] right-hand side are prepared on the host — all
O(N*D) or index work.  The device does the O(N^2 * D) dense work: for each
[128 src x 1024 dst] group, Gram matmuls (PE) -> exp (ACT) -> * C (DVE) ->
accumulating matmuls against [feat|1] (PE), then a rowwise divide.

Sharding: destination nodes are split across the 8 cores; each core computes
its npad/8 output rows end-to-end.  No collectives needed.

Engine balance (HW-measured): the kernel is paced by the C-matrix DMA
(~108us) with ACT/DVE/PE just below it.  ~24% of the exps run as a
Schraudolph bit-trick (a*x+b into int16, bitcast bf16) on the DVE to
unload the ScalarE, whose zero-depth exec queue costs ~266ns dispatch per
activation.  The all-padding source chunk is skipped (trim_pad), and the
output ships as bf16.  Rejected by measurement: fp8 matmuls in any role
(DoubleRow needs K=256; fp8 weights break the 2e-2 gate), uint8 C
transport (1-byte operands run ~3x slower on DVE), GpSimd offload
(tensor ops ~2.3us, casts ~3.9us per tile), and software pipelining lag
(the 4-deep OOO wait queue already covers the chain latency).
"""

import sys
import types

import numpy as np

try:
    from concourse import bacc, mybir, tile
    from concourse.bass_utils import run_bass_kernel_spmd
except ImportError:  # harness container may not have the repo on sys.path
    for _p in ("/opt/trn_rl_repo", "/root/.axon_site/_ro/trn_rl_repo"):
        if _p not in sys.path:
            sys.path.append(_p)
    from concourse import bacc, mybir, tile
    from concourse.bass_utils import run_bass_kernel_spmd

import ml_dtypes

F32 = mybir.dt.float32
BF16 = mybir.dt.bfloat16
FP8 = mybir.dt.float8e4
AF = mybir.ActivationFunctionType
ALU = mybir.AluOpType
DR = mybir.MatmulPerfMode.DoubleRow

D = 128  # feature dim
GFREE = 1024  # uniform group free size (gm * ck)


def _spread(n, count):
    """Pick `count` of n slots, evenly interleaved."""
    if count <= 0:
        return set()
    f = count / n
    return {g for g in range(n) if int((g + 1) * f) > int(g * f)}


def make_cfg(n_nodes=10000, npad=10240, ncores=8,
             lag=0, cb_pair=True, jfin=True, fp8_gram=False,
             fp8_plain=False, n_gpx=0, trim_pad=True,
             n_dve_exp=24, n_gp_mult=35, n_dve_u8=25, out_bf16=True,
             cast_dma=False):
    c = types.SimpleNamespace()
    c.cast_dma = cast_dma      # ship C as u8, SWDGE cast-DMA expands to bf16
    c.n_nodes = n_nodes
    c.npad = npad              # padded node count (multiple of 128*ncores)
    c.ncores = ncores
    c.fp8_gram = fp8_gram      # Gram matmuls in fp8 DoubleRow (2x PE rate)
    c.fp8_plain = fp8_plain    # Gram operands fp8 (same PE rate, half DMA)
    c.npc = npad // ncores     # dst columns per core
    c.mch = npad // 128        # source-node chunks (contraction dim)
    c.tt = c.npc // 128        # output row-tiles per core
    c.lag = lag                # groups of G->B software pipelining
    c.cb_pair = cb_pair        # 2-group C DMA batching
    c.jfin = jfin              # finalize/ship output per j-chunk
    # j-chunks of dst columns: prefer 512 wide, remainder in one chunk
    c.jchunks = []             # (joff, ck, gm, ngroups_j)
    off = 0
    while off < c.npc:
        ck = min(512, c.npc - off)
        assert ck % 128 == 0 and GFREE % ck == 0
        gm = GFREE // ck
        assert c.mch % gm == 0
        c.jchunks.append((off, ck, gm, c.mch * ck // GFREE))
        off += ck
    c.ngroups = c.mch * c.npc // GFREE
    # Per-group engine/dtype assignment, each class evenly interleaved.
    # exp: ACT activation vs DVE Schraudolph bit-trick.
    # mult: DVE tensor_tensor vs GpSimd.
    # C transport: bf16 (2x DVE mult) vs uint8 (half DMA, 1x mult; free
    # on GpSimd, whose cost is dtype-independent).
    c.n_dve_exp, c.n_gp_mult, c.n_dve_u8 = n_dve_exp, n_gp_mult, n_dve_u8
    c.n_gpx = n_gpx
    c.out_bf16 = out_bf16
    c.trim_pad = trim_pad
    # last fully-padded source chunk (nodes >= n_nodes): skip its work
    c.pad_chunk = c.mch - 1 if trim_pad and n_nodes <= (c.mch - 1) * 128 else -1
    c.dve_exp_groups = _spread(c.ngroups, n_dve_exp)
    c.gp_groups = _spread(c.ngroups, n_gp_mult)
    c.gpx_groups = _spread(c.ngroups, n_gpx) - c.gp_groups
    dve_mult = [g for g in range(c.ngroups) if g not in c.gp_groups]
    u8_dve = {dve_mult[i] for i in sorted(
        {int(j * len(dve_mult) / max(n_dve_u8, 1)) for j in range(n_dve_u8)}
    )} if n_dve_u8 else set()
    u8set = c.gp_groups | u8_dve | c.gpx_groups
    c.u8_groups = sorted(u8set)
    c.bf_groups = [g for g in range(c.ngroups) if g not in u8set]
    return c


def build(cfg):
    """Build the per-core SPMD graph (identical on all cores; data differs)."""
    nc = bacc.Bacc(
        "TRN2", target_bir_lowering=False, debug=False, num_devices=cfg.ncores
    )
    D1 = D + 1
    gdt = FP8 if cfg.fp8_plain else BF16
    if cfg.fp8_gram:
        # features split into two 64-row K-tiles for DoubleRow fp8 matmul
        ntd = nc.dram_tensor("normT", [64, 2, cfg.npad], FP8, kind="ExternalInput")
        nmd = nc.dram_tensor("normTmy", [64, 2, cfg.npc], FP8, kind="ExternalInput")
    else:
        ntd = nc.dram_tensor("normT", [128, cfg.npad], gdt, kind="ExternalInput")
        nmd = nc.dram_tensor("normTmy", [128, cfg.npc], gdt, kind="ExternalInput")
    fqd = nc.dram_tensor("featq", [128, cfg.mch * D1], BF16, kind="ExternalInput")
    n16, n8 = len(cfg.bf_groups), len(cfg.u8_groups)
    ctd = nc.dram_tensor(
        "ct", [128, n16 * GFREE],
        mybir.dt.uint8 if cfg.cast_dma else BF16, kind="ExternalInput"
    )
    ct8d = (
        nc.dram_tensor("ct8", [128, n8 * GFREE], mybir.dt.uint8,
                       kind="ExternalInput")
        if n8 else None
    )
    odt = BF16 if cfg.out_bf16 else F32
    outd = nc.dram_tensor("out", [128, cfg.tt, D], odt, kind="ExternalOutput")

    with tile.TileContext(nc) as tc:
        with (
            tc.tile_pool(name="const", bufs=1) as constp,
            tc.tile_pool(name="big", bufs=1) as bigp,
            tc.tile_pool(name="cb", bufs=5) as cbp,
            tc.tile_pool(name="cbx", bufs=6) as cbxp,
            tc.tile_pool(name="eg", bufs=8) as egp,
            tc.tile_pool(name="ht", bufs=8) as htp,
            tc.tile_pool(name="pg", bufs=3, space="PSUM") as pgp,
            tc.tile_pool(name="po", bufs=2, space="PSUM") as pop,
        ):
            if cfg.fp8_gram:
                normT = bigp.tile([64, 2, cfg.npad], FP8)
                normTmy = bigp.tile([64, 2, cfg.npc], FP8)
            else:
                normT = bigp.tile([128, cfg.npad], gdt)
                normTmy = bigp.tile([128, cfg.npc], gdt)
            featq = bigp.tile([128, cfg.mch * D1], BF16)
            outacc = bigp.tile([128, cfg.tt, D1], F32)
            final = bigp.tile([128, cfg.npc], BF16 if cfg.out_bf16 else F32)

            def nt_slice(a, b):
                return (normT[:, :, a:b], ntd[:, :, a:b]) if cfg.fp8_gram else (
                    normT[:, a:b], ntd[:, a:b])

            # group-0 blockers first: first normT slice, first nmy j-slice
            nc.sync.dma_start(*nt_slice(0, 256))
            if cfg.fp8_gram:
                nc.sync.dma_start(normTmy[:], nmd[:])
            else:
                nc.sync.dma_start(normTmy[:, 0:512], nmd[:, 0:512])
                for a, b in ((512, 1024), (1024, cfg.npc)):
                    if b > a:
                        nc.sync.dma_start(normTmy[:, a:b], nmd[:, a:b])
            nsplit = 8
            stepn = max(128, (cfg.npad // nsplit) // 128 * 128)
            stepq = max(D1, (cfg.mch * D1 // nsplit) // D1 * D1)
            qoffs = list(range(0, cfg.mch * D1, stepq))
            noffs = list(range(0, cfg.npad, stepn))
            cbw = 2 if cfg.cb_pair else 1
            pos16 = {g: i for i, g in enumerate(cfg.bf_groups)}
            pos8 = {g: i for i, g in enumerate(cfg.u8_groups)}
            cb_tiles = {"16": {}, "8": {}}
            exp8 = {}

            def fetch_cb_pair(mod, pix):
                # one DMA covering cbw consecutive same-modality groups
                if mod == "16":
                    total, dram, dt, tg = len(cfg.bf_groups) * GFREE, ctd, BF16, "cb"
                else:
                    total, dram, dt, tg = (
                        len(cfg.u8_groups) * GFREE, ct8d, mybir.dt.uint8, "cb8"
                    )
                lo = pix * cbw * GFREE
                hi = min((pix * cbw + cbw) * GFREE, total)
                cbt = cbp.tile([128, cbw * GFREE], dt, tag=tg, name=f"cb{mod}")
                if mod == "16" and cfg.cast_dma:
                    # C travels as u8 in HBM; the SWDGE datapath widens to
                    # bf16 on the SBUF write side (halves HBM-side traffic)
                    nc.gpsimd.dma_start(cbt[:, 0 : hi - lo], dram[:, lo:hi])
                else:
                    nc.sync.dma_start(cbt[:, 0 : hi - lo], dram[:, lo:hi])
                cb_tiles[mod][pix] = cbt
                if mod == "8":
                    # u8 -> bf16 expansion on the (otherwise idle) GpSimd,
                    # off the critical path: depends only on the C DMA
                    for h in range((hi - lo) // GFREE):
                        g8 = cfg.u8_groups[pix * cbw + h]
                        if g8 in cfg.gpx_groups:
                            xt = cbxp.tile([128, GFREE], BF16, tag="cbx",
                                           name="cbx")
                            nc.gpsimd.tensor_copy(
                                xt[:], cbt[:, h * GFREE : (h + 1) * GFREE]
                            )
                            exp8[g8] = xt

            def get_cb(g):
                mod = "8" if g in pos8 else "16"
                pos = pos8[g] if mod == "8" else pos16[g]
                pix = pos // cbw
                if pix not in cb_tiles[mod]:
                    fetch_cb_pair(mod, pix)
                if mod == "8" and (pix + 1) * cbw < len(cfg.u8_groups) and (
                    pix + 1
                ) not in cb_tiles[mod]:
                    # lookahead so GpSimd expansion leads the consumer
                    fetch_cb_pair(mod, pix + 1)
                t = cb_tiles[mod][pix]
                if pos % cbw == cbw - 1 or g == cfg.ngroups - 1:
                    cb_tiles[mod].pop(pix)
                if g in cfg.gpx_groups:
                    return exp8.pop(g)[:]
                return t[:, (pos % cbw) * GFREE : (pos % cbw + 1) * GFREE]

            nc.sync.dma_start(featq[:, 0 : 8 * D1], fqd[:, 0 : 8 * D1])
            for ix in range(max(len(qoffs), len(noffs))):
                if ix < 3:  # stream C from t=0
                    if ix * cbw < len(cfg.bf_groups):
                        fetch_cb_pair("16", ix)
                    if ix * cbw < len(cfg.u8_groups):
                        fetch_cb_pair("8", ix)
                if ix < len(noffs):
                    a = max(noffs[ix], 256 if ix == 0 else 0)
                    b = min(noffs[ix] + stepn, cfg.npad)
                    if b > a:
                        nc.sync.dma_start(*nt_slice(a, b))
                if ix < len(qoffs):
                    a = qoffs[ix] + (8 * D1 if ix == 0 else 0)
                    b = min(qoffs[ix] + stepq, cfg.mch * D1)
                    if b > a:
                        nc.sync.dma_start(featq[:, a:b], fqd[:, a:b])

            dmax = constp.tile([128, cfg.tt], F32)
            rden = constp.tile([128, cfg.tt], F32)

            # ---- main loop: uniform [128, GFREE] groups, software-
            # pipelined so a group's B-matmuls trail its G-matmuls by
            # cfg.lag groups (hides the psum->exp->mult latency on PE) ----
            groups = []  # flat (joff, ck, gm, ns, gj, ngj, jix)
            for jix, (joff, ck, gm, ngj) in enumerate(cfg.jchunks):
                for gj in range(ngj):
                    groups.append((joff, ck, gm, ck // 128, gj, ngj, jix))
            po_by_j = {}
            ht_by_g = {}

            def emit_front(gidx):
                joff, ck, gm, ns, gj, ngj, jix = groups[gidx]
                if jix not in po_by_j:
                    po_by_j[jix] = [
                        pop.tile([128, 2 * D1], F32, tag="po", name=f"po{jix}_{u}")
                        for u in range((ns + 1) // 2)
                    ]
                pg = pgp.tile([128, GFREE], F32, tag="pg", name="pg")
                kuse = gm - (1 if gj * gm + gm - 1 == cfg.pad_chunk else 0)
                fr = kuse * ck  # active free size (pad chunk trimmed)
                for k in range(kuse):
                    i = gj * gm + k
                    if cfg.fp8_gram:
                        nc.tensor.matmul(
                            pg[:, k * ck : (k + 1) * ck],
                            normT[:, :, i * 128 : (i + 1) * 128],
                            normTmy[:, :, joff : joff + ck],
                            start=True, stop=True, perf_mode=DR,
                        )
                    else:
                        nc.tensor.matmul(
                            pg[:, k * ck : (k + 1) * ck],
                            normT[:, i * 128 : (i + 1) * 128],
                            normTmy[:, joff : joff + ck],
                            start=True, stop=True,
                        )
                cb = get_cb(gidx)
                mult_eng = (
                    nc.gpsimd if gidx in cfg.gp_groups else nc.vector
                )
                ht = htp.tile([128, GFREE], BF16, tag="ht", name="ht")
                if gidx in cfg.dve_exp_groups:
                    # exp(x) ~= bf16_bits(round(184.665*x + 16250.4)):
                    # Schraudolph bit-trick on DVE, offloading ScalarE
                    si = egp.tile(
                        [128, GFREE], mybir.dt.int16, tag="eg", name="si"
                    )
                    nc.vector.tensor_scalar(
                        out=si[:, 0:fr], in0=pg[:, 0:fr], scalar1=184.664965,
                        scalar2=16250.4, op0=ALU.mult, op1=ALU.add,
                    )
                    mult_eng.tensor_tensor(
                        ht[:, 0:fr], si[:, 0:fr].bitcast(BF16), cb[:, 0:fr],
                        op=ALU.mult,
                    )
                else:
                    eg = egp.tile([128, GFREE], BF16, tag="eg", name="eg")
                    nc.scalar.activation(eg[:, 0:fr], pg[:, 0:fr], AF.Exp)
                    mult_eng.tensor_tensor(
                        ht[:, 0:fr], eg[:, 0:fr], cb[:, 0:fr], op=ALU.mult
                    )
                ht_by_g[gidx] = ht

            def emit_back(gidx):
                joff, ck, gm, ns, gj, ngj, jix = groups[gidx]
                ht = ht_by_g.pop(gidx)
                po = po_by_j[jix]
                kuse = gm - (1 if gj * gm + gm - 1 == cfg.pad_chunk else 0)
                for k in range(kuse):
                    i = gj * gm + k
                    for s in range(ns):
                        nc.tensor.matmul(
                            po[s // 2][:, (s % 2) * D1 : (s % 2 + 1) * D1],
                            ht[:, k * ck + s * 128 : k * ck + (s + 1) * 128],
                            featq[:, i * D1 : (i + 1) * D1],
                            # start zeroes the whole 2KB PSUM bank, so only
                            # the first region of each packed pair sets it
                            start=(gj == 0 and k == 0 and s % 2 == 0),
                            stop=(gj == ngj - 1 and k == kuse - 1),
                            skip_group_check=True,
                        )
                if gj == ngj - 1:  # last group of this j-chunk: drain po,
                    t0 = joff // 128   # divide and ship this slice out now
                    for s in range(ns):
                        nc.vector.tensor_copy(
                            outacc[:, t0 + s, :],
                            po[s // 2][:, (s % 2) * D1 : (s % 2 + 1) * D1],
                        )
                    if cfg.jfin:
                        finalize_j(joff, ns)

            def finalize_j(joff, ns):
                    t0 = joff // 128
                    nc.vector.tensor_scalar(
                        out=dmax[:, t0 : t0 + ns],
                        in0=outacc[:, t0 : t0 + ns, D : D + 1],
                        scalar1=1e-30, scalar2=None, op0=ALU.max,
                    )
                    nc.vector.reciprocal(
                        rden[:, t0 : t0 + ns], dmax[:, t0 : t0 + ns]
                    )
                    for s in range(ns):
                        t = t0 + s
                        nc.vector.tensor_scalar(
                            out=final[:, t * D : (t + 1) * D],
                            in0=outacc[:, t, 0:D],
                            scalar1=rden[:, t : t + 1], scalar2=None,
                            op0=ALU.mult,
                        )
                    nc.sync.dma_start(
                        outd[:, t0 : t0 + ns, :],
                        final[:, t0 * D : (t0 + ns) * D].rearrange(
                            "p (t d) -> p t d", d=D
                        ),
                    )

            for g in range(cfg.ngroups + cfg.lag):
                if g < cfg.ngroups:
                    emit_front(g)
                if g >= cfg.lag:
                    emit_back(g - cfg.lag)
            if not cfg.jfin:
                for joff, ck, gm, ngj in cfg.jchunks:
                    finalize_j(joff, ck // 128)

    nc.compile()
    return nc


def prepare_inputs(feat, src, dst, beta, cfg):
    feat = np.ascontiguousarray(np.asarray(feat), dtype=np.float32)
    src = np.asarray(src).astype(np.int64)
    dst = np.asarray(dst).astype(np.int64)
    beta = np.asarray(beta, dtype=np.float32).reshape(-1)
    D1 = D + 1

    featp = np.zeros((cfg.npad, D), np.float32)
    featp[: cfg.n_nodes] = feat
    rn = 1.0 / np.maximum(np.linalg.norm(featp, axis=1, keepdims=True), 1e-12)
    normp = featp * rn
    if cfg.fp8_gram:
        # [64, 2, npad]: feature rows split into two 64-row K-tiles
        normT = np.ascontiguousarray(
            normp.T.reshape(2, 64, cfg.npad).transpose(1, 0, 2)
            .astype(ml_dtypes.float8_e4m3fn)
        )
    elif cfg.fp8_plain:
        normT = np.ascontiguousarray(normp.T.astype(ml_dtypes.float8_e4m3fn))
    else:
        normT = np.ascontiguousarray(normp.T.astype(ml_dtypes.bfloat16))

    # featq: [128, mch*(D+1)] bf16; block i col D holds the bias 1.0
    fq = np.ones((128, cfg.mch, D1), dtype=ml_dtypes.bfloat16)
    fq[:, :, :D] = (
        featp.astype(ml_dtypes.bfloat16).reshape(cfg.mch, 128, D).transpose(1, 0, 2)
    )
    fq = np.ascontiguousarray(fq.reshape(128, cfg.mch * D1))

    in_maps = []
    for c in range(cfg.ncores):
        lo = c * cfg.npc
        nmyT = (beta[0] * normp[lo : lo + cfg.npc]).T  # [128, npc]
        if cfg.fp8_gram:
            nmy = np.ascontiguousarray(
                nmyT.reshape(2, 64, cfg.npc).transpose(1, 0, 2)
                .astype(ml_dtypes.float8_e4m3fn)
            )
        elif cfg.fp8_plain:
            nmy = np.ascontiguousarray(nmyT.astype(ml_dtypes.float8_e4m3fn))
        else:
            nmy = np.ascontiguousarray(nmyT.astype(ml_dtypes.bfloat16))
        m = (dst >= lo) & (dst < lo + cfg.npc)
        s_c = src[m]
        d_c = dst[m] - lo
        cnt = np.bincount(
            s_c * cfg.npc + d_c, minlength=cfg.npad * cfg.npc
        ).reshape(cfg.npad, cfg.npc)
        # group-major C layout: per j-chunk, per group: [128, gm*ck]
        blocks = []
        for joff, ck, gm, ngj in cfg.jchunks:
            blk = cnt[:, joff : joff + ck].reshape(ngj, gm, 128, ck)
            blocks.append(blk.transpose(2, 0, 1, 3).reshape(128, ngj * gm * ck))
        ctall = np.concatenate(blocks, axis=1)
        gb = ctall.reshape(128, cfg.ngroups, GFREE)
        if cfg.cast_dma:
            assert ctall.max() <= 255
            ct = np.ascontiguousarray(
                gb[:, cfg.bf_groups, :].reshape(128, -1).astype(np.uint8)
            )
        else:
            ct = np.ascontiguousarray(
                gb[:, cfg.bf_groups, :].reshape(128, -1).astype(ml_dtypes.bfloat16)
            )
        im = {"normT": normT, "normTmy": nmy, "featq": fq, "ct": ct}
        if cfg.u8_groups:
            assert ctall.max() <= 255
            im["ct8"] = np.ascontiguousarray(
                gb[:, cfg.u8_groups, :].reshape(128, -1).astype(np.uint8)
            )
        in_maps.append(im)
    return in_maps


def postprocess(results, cfg):
    parts = []
    for c in range(cfg.ncores):
        o = np.asarray(results[c]["out"], np.float32)  # [128, tt, D]
        parts.append(o.transpose(1, 0, 2).reshape(cfg.npc, D))
    return np.concatenate(parts, axis=0)[: cfg.n_nodes]


_CACHE = {}


def _get_nc(cfg):
    key = (cfg.npad, cfg.ncores, cfg.n_dve_exp, cfg.n_gp_mult, cfg.lag,
           cfg.cb_pair, cfg.jfin, tuple(cfg.u8_groups), cfg.fp8_gram,
           cfg.out_bf16, cfg.fp8_plain, cfg.n_gpx, cfg.pad_chunk,
           cfg.cast_dma)
    if key not in _CACHE:
        _CACHE[key] = build(cfg)
    return _CACHE[key]


def kernel(feat, src, dst, beta):
    cfg = make_cfg()
    nc = _get_nc(cfg)
    in_maps = prepare_inputs(feat, src, dst, beta, cfg)
    res = run_bass_kernel_spmd(nc, in_maps, core_ids=list(range(cfg.ncores)))
    return postprocess(res.results, cfg)



# revision 8
# speedup vs baseline: 1.0412x; 1.0412x over previous
"""AGNNConv on 8 Trainium2 NeuronCores — dense matmul formulation.

The per-edge attention weight exp(beta * cos(src, dst)) depends only on the
(src, dst) node pair, so the whole message passing collapses to dense algebra:

    G = norm^T norm                  (Gram matrix of L2-normalized features)
    H = C  *  exp(beta * G)          (C = dense dst-by-src edge-count matrix)
    num|den = H^T @ [feat | 1]  ;    out = num / den  rowwise

The count matrix C (dense, from the edge list), the L2-normalized transposed
features, and the [feat | 1] right-hand side are prepared on the host — all
O(N*D) or index work.  The device does the O(N^2 * D) dense work: for each
[128 src x 1024 dst] group, Gram matmuls (PE) -> exp (ACT) -> * C (DVE) ->
accumulating matmuls against [feat|1] (PE), then a rowwise divide.

Sharding: destination nodes are split across the 8 cores; each core computes
its npad/8 output rows end-to-end.  No collectives needed.

Engine balance (HW-measured): the kernel is paced by the C-matrix DMA
(~108us) with ACT/DVE/PE just below it.  ~24% of the exps run as a
Schraudolph bit-trick (a*x+b into int16, bitcast bf16) on the DVE to
unload the ScalarE, whose zero-depth exec queue costs ~266ns dispatch per
activation.  The all-padding source chunk is skipped (trim_pad), and the
output ships as bf16.  Rejected by measurement: fp8 matmuls in any role
(DoubleRow needs K=256; fp8 weights break the 2e-2 gate), uint8 C
transport (1-byte operands run ~3x slower on DVE), GpSimd offload
(tensor ops ~2.3us, casts ~3.9us per tile), and software pipelining lag
(the 4-deep OOO wait queue already covers the chain latency).
"""

import sys
import types

import numpy as np

try:
    from concourse import bacc, mybir, tile
    from concourse.bass_utils import run_bass_kernel_spmd
except ImportError:  # harness container may not have the repo on sys.path
    for _p in ("/opt/trn_rl_repo", "/root/.axon_site/_ro/trn_rl_repo"):
        if _p not in sys.path:
            sys.path.append(_p)
    from concourse import bacc, mybir, tile
    from concourse.bass_utils import run_bass_kernel_spmd

import ml_dtypes

F32 = mybir.dt.float32
BF16 = mybir.dt.bfloat16
FP8 = mybir.dt.float8e4
AF = mybir.ActivationFunctionType
ALU = mybir.AluOpType
DR = mybir.MatmulPerfMode.DoubleRow

D = 128  # feature dim
GFREE = 1024  # uniform group free size (gm * ck)


def _spread(n, count):
    """Pick `count` of n slots, evenly interleaved."""
    if count <= 0:
        return set()
    f = count / n
    return {g for g in range(n) if int((g + 1) * f) > int(g * f)}


def make_cfg(n_nodes=10000, npad=10240, ncores=8,
             lag=0, cb_pair=True, jfin=True, fp8_gram=False,
             fp8_plain=False, n_gpx=0, trim_pad=True,
             n_dve_exp=24, n_gp_mult=35, n_dve_u8=0, out_bf16=True,
             cast_dma=False):
    c = types.SimpleNamespace()
    c.cast_dma = cast_dma      # ship C as u8, SWDGE cast-DMA expands to bf16
    c.n_nodes = n_nodes
    c.npad = npad              # padded node count (multiple of 128*ncores)
    c.ncores = ncores
    c.fp8_gram = fp8_gram      # Gram matmuls in fp8 DoubleRow (2x PE rate)
    c.fp8_plain = fp8_plain    # Gram operands fp8 (same PE rate, half DMA)
    c.npc = npad // ncores     # dst columns per core
    c.mch = npad // 128        # source-node chunks (contraction dim)
    c.tt = c.npc // 128        # output row-tiles per core
    c.lag = lag                # groups of G->B software pipelining
    c.cb_pair = cb_pair        # 2-group C DMA batching
    c.jfin = jfin              # finalize/ship output per j-chunk
    # j-chunks of dst columns: prefer 512 wide, remainder in one chunk
    c.jchunks = []             # (joff, ck, gm, ngroups_j)
    off = 0
    while off < c.npc:
        ck = min(512, c.npc - off)
        assert ck % 128 == 0 and GFREE % ck == 0
        gm = GFREE // ck
        assert c.mch % gm == 0
        c.jchunks.append((off, ck, gm, c.mch * ck // GFREE))
        off += ck
    c.ngroups = c.mch * c.npc // GFREE
    # Per-group engine/dtype assignment, each class evenly interleaved.
    # exp: ACT activation vs DVE Schraudolph bit-trick.
    # mult: DVE tensor_tensor vs GpSimd.
    # C transport: bf16 (2x DVE mult) vs uint8 (half DMA, 1x mult; free
    # on GpSimd, whose cost is dtype-independent).
    c.n_dve_exp, c.n_gp_mult, c.n_dve_u8 = n_dve_exp, n_gp_mult, n_dve_u8
    c.n_gpx = n_gpx
    c.out_bf16 = out_bf16
    c.trim_pad = trim_pad
    # last fully-padded source chunk (nodes >= n_nodes): skip its work
    c.pad_chunk = c.mch - 1 if trim_pad and n_nodes <= (c.mch - 1) * 128 else -1
    c.dve_exp_groups = _spread(c.ngroups, n_dve_exp)
    c.gp_groups = _spread(c.ngroups, n_gp_mult)
    c.gpx_groups = _spread(c.ngroups, n_gpx) - c.gp_groups
    dve_mult = [g for g in range(c.ngroups) if g not in c.gp_groups]
    u8_dve = {dve_mult[i] for i in sorted(
        {int(j * len(dve_mult) / max(n_dve_u8, 1)) for j in range(n_dve_u8)}
    )} if n_dve_u8 else set()
    u8set = c.gp_groups | u8_dve | c.gpx_groups
    c.u8_groups = sorted(u8set)
    c.bf_groups = [g for g in range(c.ngroups) if g not in u8set]
    return c


def build(cfg):
    """Build the per-core SPMD graph (identical on all cores; data differs)."""
    nc = bacc.Bacc(
        "TRN2", target_bir_lowering=False, debug=False, num_devices=cfg.ncores
    )
    D1 = D + 1
    gdt = FP8 if cfg.fp8_plain else BF16
    if cfg.fp8_gram:
        # features split into two 64-row K-tiles for DoubleRow fp8 matmul
        ntd = nc.dram_tensor("normT", [64, 2, cfg.npad], FP8, kind="ExternalInput")
        nmd = nc.dram_tensor("normTmy", [64, 2, cfg.npc], FP8, kind="ExternalInput")
    else:
        ntd = nc.dram_tensor("normT", [128, cfg.npad], gdt, kind="ExternalInput")
        nmd = nc.dram_tensor("normTmy", [128, cfg.npc], gdt, kind="ExternalInput")
    fqd = nc.dram_tensor("featq", [128, cfg.mch * D1], BF16, kind="ExternalInput")
    n16, n8 = len(cfg.bf_groups), len(cfg.u8_groups)
    ctd = nc.dram_tensor(
        "ct", [128, n16 * GFREE],
        mybir.dt.uint8 if cfg.cast_dma else BF16, kind="ExternalInput"
    )
    ct8d = (
        nc.dram_tensor("ct8", [128, n8 * GFREE], mybir.dt.uint8,
                       kind="ExternalInput")
        if n8 else None
    )
    odt = BF16 if cfg.out_bf16 else F32
    outd = nc.dram_tensor("out", [128, cfg.tt, D], odt, kind="ExternalOutput")

    with tile.TileContext(nc) as tc:
        with (
            tc.tile_pool(name="const", bufs=1) as constp,
            tc.tile_pool(name="big", bufs=1) as bigp,
            tc.tile_pool(name="cb", bufs=5) as cbp,
            tc.tile_pool(name="cbx", bufs=6) as cbxp,
            tc.tile_pool(name="eg", bufs=8) as egp,
            tc.tile_pool(name="ht", bufs=8) as htp,
            tc.tile_pool(name="pg", bufs=3, space="PSUM") as pgp,
            tc.tile_pool(name="po", bufs=2, space="PSUM") as pop,
        ):
            if cfg.fp8_gram:
                normT = bigp.tile([64, 2, cfg.npad], FP8)
                normTmy = bigp.tile([64, 2, cfg.npc], FP8)
            else:
                normT = bigp.tile([128, cfg.npad], gdt)
                normTmy = bigp.tile([128, cfg.npc], gdt)
            featq = bigp.tile([128, cfg.mch * D1], BF16)
            outacc = bigp.tile([128, cfg.tt, D1], F32)
            final = bigp.tile([128, cfg.npc], BF16 if cfg.out_bf16 else F32)

            def nt_slice(a, b):
                return (normT[:, :, a:b], ntd[:, :, a:b]) if cfg.fp8_gram else (
                    normT[:, a:b], ntd[:, a:b])

            # group-0 blockers first: first normT slice, first nmy j-slice
            nc.sync.dma_start(*nt_slice(0, 256))
            if cfg.fp8_gram:
                nc.sync.dma_start(normTmy[:], nmd[:])
            else:
                nc.sync.dma_start(normTmy[:, 0:512], nmd[:, 0:512])
                for a, b in ((512, 1024), (1024, cfg.npc)):
                    if b > a:
                        nc.sync.dma_start(normTmy[:, a:b], nmd[:, a:b])
            nsplit = 8
            stepn = max(128, (cfg.npad // nsplit) // 128 * 128)
            stepq = max(D1, (cfg.mch * D1 // nsplit) // D1 * D1)
            qoffs = list(range(0, cfg.mch * D1, stepq))
            noffs = list(range(0, cfg.npad, stepn))
            cbw = 2 if cfg.cb_pair else 1
            pos16 = {g: i for i, g in enumerate(cfg.bf_groups)}
            pos8 = {g: i for i, g in enumerate(cfg.u8_groups)}
            cb_tiles = {"16": {}, "8": {}}
            exp8 = {}

            def fetch_cb_pair(mod, pix):
                # one DMA covering cbw consecutive same-modality groups
                if mod == "16":
                    total, dram, dt, tg = len(cfg.bf_groups) * GFREE, ctd, BF16, "cb"
                else:
                    total, dram, dt, tg = (
                        len(cfg.u8_groups) * GFREE, ct8d, mybir.dt.uint8, "cb8"
                    )
                lo = pix * cbw * GFREE
                hi = min((pix * cbw + cbw) * GFREE, total)
                cbt = cbp.tile([128, cbw * GFREE], dt, tag=tg, name=f"cb{mod}")
                if mod == "16" and cfg.cast_dma:
                    # C travels as u8 in HBM; the SWDGE datapath widens to
                    # bf16 on the SBUF write side (halves HBM-side traffic)
                    nc.gpsimd.dma_start(cbt[:, 0 : hi - lo], dram[:, lo:hi])
                else:
                    nc.sync.dma_start(cbt[:, 0 : hi - lo], dram[:, lo:hi])
                cb_tiles[mod][pix] = cbt
                if mod == "8":
                    # u8 -> bf16 expansion on the (otherwise idle) GpSimd,
                    # off the critical path: depends only on the C DMA
                    for h in range((hi - lo) // GFREE):
                        g8 = cfg.u8_groups[pix * cbw + h]
                        if g8 in cfg.gpx_groups:
                            xt = cbxp.tile([128, GFREE], BF16, tag="cbx",
                                           name="cbx")
                            nc.gpsimd.tensor_copy(
                                xt[:], cbt[:, h * GFREE : (h + 1) * GFREE]
                            )
                            exp8[g8] = xt

            def get_cb(g):
                mod = "8" if g in pos8 else "16"
                pos = pos8[g] if mod == "8" else pos16[g]
                pix = pos // cbw
                if pix not in cb_tiles[mod]:
                    fetch_cb_pair(mod, pix)
                if mod == "8" and (pix + 1) * cbw < len(cfg.u8_groups) and (
                    pix + 1
                ) not in cb_tiles[mod]:
                    # lookahead so GpSimd expansion leads the consumer
                    fetch_cb_pair(mod, pix + 1)
                t = cb_tiles[mod][pix]
                if pos % cbw == cbw - 1 or g == cfg.ngroups - 1:
                    cb_tiles[mod].pop(pix)
                if g in cfg.gpx_groups:
                    return exp8.pop(g)[:]
                return t[:, (pos % cbw) * GFREE : (pos % cbw + 1) * GFREE]

            nc.sync.dma_start(featq[:, 0 : 8 * D1], fqd[:, 0 : 8 * D1])
            for ix in range(max(len(qoffs), len(noffs))):
                if ix < 3:  # stream C from t=0
                    if ix * cbw < len(cfg.bf_groups):
                        fetch_cb_pair("16", ix)
                    if ix * cbw < len(cfg.u8_groups):
                        fetch_cb_pair("8", ix)
                if ix < len(noffs):
                    a = max(noffs[ix], 256 if ix == 0 else 0)
                    b = min(noffs[ix] + stepn, cfg.npad)
                    if b > a:
                        nc.sync.dma_start(*nt_slice(a, b))
                if ix < len(qoffs):
                    a = qoffs[ix] + (8 * D1 if ix == 0 else 0)
                    b = min(qoffs[ix] + stepq, cfg.mch * D1)
                    if b > a:
                        nc.sync.dma_start(featq[:, a:b], fqd[:, a:b])

            dmax = constp.tile([128, cfg.tt], F32)
            rden = constp.tile([128, cfg.tt], F32)

            # ---- main loop: uniform [128, GFREE] groups, software-
            # pipelined so a group's B-matmuls trail its G-matmuls by
            # cfg.lag groups (hides the psum->exp->mult latency on PE) ----
            groups = []  # flat (joff, ck, gm, ns, gj, ngj, jix)
            for jix, (joff, ck, gm, ngj) in enumerate(cfg.jchunks):
                for gj in range(ngj):
                    groups.append((joff, ck, gm, ck // 128, gj, ngj, jix))
            po_by_j = {}
            ht_by_g = {}

            def emit_front(gidx):
                joff, ck, gm, ns, gj, ngj, jix = groups[gidx]
                if jix not in po_by_j:
                    po_by_j[jix] = [
                        pop.tile([128, 2 * D1], F32, tag="po", name=f"po{jix}_{u}")
                        for u in range((ns + 1) // 2)
                    ]
                pg = pgp.tile([128, GFREE], F32, tag="pg", name="pg")
                kuse = gm - (1 if gj * gm + gm - 1 == cfg.pad_chunk else 0)
                fr = kuse * ck  # active free size (pad chunk trimmed)
                for k in range(kuse):
                    i = gj * gm + k
                    if cfg.fp8_gram:
                        nc.tensor.matmul(
                            pg[:, k * ck : (k + 1) * ck],
                            normT[:, :, i * 128 : (i + 1) * 128],
                            normTmy[:, :, joff : joff + ck],
                            start=True, stop=True, perf_mode=DR,
                        )
                    else:
                        nc.tensor.matmul(
                            pg[:, k * ck : (k + 1) * ck],
                            normT[:, i * 128 : (i + 1) * 128],
                            normTmy[:, joff : joff + ck],
                            start=True, stop=True,
                        )
                cb = get_cb(gidx)
                mult_eng = (
                    nc.gpsimd if gidx in cfg.gp_groups else nc.vector
                )
                ht = htp.tile([128, GFREE], BF16, tag="ht", name="ht")
                if gidx in cfg.dve_exp_groups:
                    # exp(x) ~= bf16_bits(round(184.665*x + 16250.4)):
                    # Schraudolph bit-trick on DVE, offloading ScalarE
                    si = egp.tile(
                        [128, GFREE], mybir.dt.int16, tag="eg", name="si"
                    )
                    nc.vector.tensor_scalar(
                        out=si[:, 0:fr], in0=pg[:, 0:fr], scalar1=184.664965,
                        scalar2=16250.4, op0=ALU.mult, op1=ALU.add,
                    )
                    mult_eng.tensor_tensor(
                        ht[:, 0:fr], si[:, 0:fr].bitcast(BF16), cb[:, 0:fr],
                        op=ALU.mult,
                    )
                else:
                    eg = egp.tile([128, GFREE], BF16, tag="eg", name="eg")
                    nc.scalar.activation(eg[:, 0:fr], pg[:, 0:fr], AF.Exp)
                    mult_eng.tensor_tensor(
                        ht[:, 0:fr], eg[:, 0:fr], cb[:, 0:fr], op=ALU.mult
                    )
                ht_by_g[gidx] = ht

            def emit_back(gidx):
                joff, ck, gm, ns, gj, ngj, jix = groups[gidx]
                ht = ht_by_g.pop(gidx)
                po = po_by_j[jix]
                kuse = gm - (1 if gj * gm + gm - 1 == cfg.pad_chunk else 0)
                for k in range(kuse):
                    i = gj * gm + k
                    for s in range(ns):
                        nc.tensor.matmul(
                            po[s // 2][:, (s % 2) * D1 : (s % 2 + 1) * D1],
                            ht[:, k * ck + s * 128 : k * ck + (s + 1) * 128],
                            featq[:, i * D1 : (i + 1) * D1],
                            # start zeroes the whole 2KB PSUM bank, so only
                            # the first region of each packed pair sets it
                            start=(gj == 0 and k == 0 and s % 2 == 0),
                            stop=(gj == ngj - 1 and k == kuse - 1),
                            skip_group_check=True,
                        )
                if gj == ngj - 1:  # last group of this j-chunk: drain po,
                    t0 = joff // 128   # divide and ship this slice out now
                    for s in range(ns):
                        nc.vector.tensor_copy(
                            outacc[:, t0 + s, :],
                            po[s // 2][:, (s % 2) * D1 : (s % 2 + 1) * D1],
                        )
                    if cfg.jfin:
                        finalize_j(joff, ns)

            def finalize_j(joff, ns):
                    t0 = joff // 128
                    nc.vector.tensor_scalar(
                        out=dmax[:, t0 : t0 + ns],
                        in0=outacc[:, t0 : t0 + ns, D : D + 1],
                        scalar1=1e-30, scalar2=None, op0=ALU.max,
                    )
                    nc.vector.reciprocal(
                        rden[:, t0 : t0 + ns], dmax[:, t0 : t0 + ns]
                    )
                    for s in range(ns):
                        t = t0 + s
                        nc.vector.tensor_scalar(
                            out=final[:, t * D : (t + 1) * D],
                            in0=outacc[:, t, 0:D],
                            scalar1=rden[:, t : t + 1], scalar2=None,
                            op0=ALU.mult,
                        )
                    nc.sync.dma_start(
                        outd[:, t0 : t0 + ns, :],
                        final[:, t0 * D : (t0 + ns) * D].rearrange(
                            "p (t d) -> p t d", d=D
                        ),
                    )

            for g in range(cfg.ngroups + cfg.lag):
                if g < cfg.ngroups:
                    emit_front(g)
                if g >= cfg.lag:
                    emit_back(g - cfg.lag)
            if not cfg.jfin:
                for joff, ck, gm, ngj in cfg.jchunks:
                    finalize_j(joff, ck // 128)

    nc.compile()
    return nc


def prepare_inputs(feat, src, dst, beta, cfg):
    feat = np.ascontiguousarray(np.asarray(feat), dtype=np.float32)
    src = np.asarray(src).astype(np.int64)
    dst = np.asarray(dst).astype(np.int64)
    beta = np.asarray(beta, dtype=np.float32).reshape(-1)
    D1 = D + 1

    featp = np.zeros((cfg.npad, D), np.float32)
    featp[: cfg.n_nodes] = feat
    rn = 1.0 / np.maximum(np.linalg.norm(featp, axis=1, keepdims=True), 1e-12)
    normp = featp * rn
    if cfg.fp8_gram:
        # [64, 2, npad]: feature rows split into two 64-row K-tiles
        normT = np.ascontiguousarray(
            normp.T.reshape(2, 64, cfg.npad).transpose(1, 0, 2)
            .astype(ml_dtypes.float8_e4m3fn)
        )
    elif cfg.fp8_plain:
        normT = np.ascontiguousarray(normp.T.astype(ml_dtypes.float8_e4m3fn))
    else:
        normT = np.ascontiguousarray(normp.T.astype(ml_dtypes.bfloat16))

    # featq: [128, mch*(D+1)] bf16; block i col D holds the bias 1.0
    fq = np.ones((128, cfg.mch, D1), dtype=ml_dtypes.bfloat16)
    fq[:, :, :D] = (
        featp.astype(ml_dtypes.bfloat16).reshape(cfg.mch, 128, D).transpose(1, 0, 2)
    )
    fq = np.ascontiguousarray(fq.reshape(128, cfg.mch * D1))

    in_maps = []
    for c in range(cfg.ncores):
        lo = c * cfg.npc
        nmyT = (beta[0] * normp[lo : lo + cfg.npc]).T  # [128, npc]
        if cfg.fp8_gram:
            nmy = np.ascontiguousarray(
                nmyT.reshape(2, 64, cfg.npc).transpose(1, 0, 2)
                .astype(ml_dtypes.float8_e4m3fn)
            )
        elif cfg.fp8_plain:
            nmy = np.ascontiguousarray(nmyT.astype(ml_dtypes.float8_e4m3fn))
        else:
            nmy = np.ascontiguousarray(nmyT.astype(ml_dtypes.bfloat16))
        m = (dst >= lo) & (dst < lo + cfg.npc)
        s_c = src[m]
        d_c = dst[m] - lo
        cnt = np.bincount(
            s_c * cfg.npc + d_c, minlength=cfg.npad * cfg.npc
        ).reshape(cfg.npad, cfg.npc)
        # group-major C layout: per j-chunk, per group: [128, gm*ck]
        blocks = []
        for joff, ck, gm, ngj in cfg.jchunks:
            blk = cnt[:, joff : joff + ck].reshape(ngj, gm, 128, ck)
            blocks.append(blk.transpose(2, 0, 1, 3).reshape(128, ngj * gm * ck))
        ctall = np.concatenate(blocks, axis=1)
        gb = ctall.reshape(128, cfg.ngroups, GFREE)
        if cfg.cast_dma:
            assert ctall.max() <= 255
            ct = np.ascontiguousarray(
                gb[:, cfg.bf_groups, :].reshape(128, -1).astype(np.uint8)
            )
        else:
            ct = np.ascontiguousarray(
                gb[:, cfg.bf_groups, :].reshape(128, -1).astype(ml_dtypes.bfloat16)
            )
        im = {"normT": normT, "normTmy": nmy, "featq": fq, "ct": ct}
        if cfg.u8_groups:
            assert ctall.max() <= 255
            im["ct8"] = np.ascontiguousarray(
                gb[:, cfg.u8_groups, :].reshape(128, -1).astype(np.uint8)
            )
        in_maps.append(im)
    return in_maps


def postprocess(results, cfg):
    parts = []
    for c in range(cfg.ncores):
        o = np.asarray(results[c]["out"], np.float32)  # [128, tt, D]
        parts.append(o.transpose(1, 0, 2).reshape(cfg.npc, D))
    return np.concatenate(parts, axis=0)[: cfg.n_nodes]


_CACHE = {}


def _get_nc(cfg):
    key = (cfg.npad, cfg.ncores, cfg.n_dve_exp, cfg.n_gp_mult, cfg.lag,
           cfg.cb_pair, cfg.jfin, tuple(cfg.u8_groups), cfg.fp8_gram,
           cfg.out_bf16, cfg.fp8_plain, cfg.n_gpx, cfg.pad_chunk,
           cfg.cast_dma)
    if key not in _CACHE:
        _CACHE[key] = build(cfg)
    return _CACHE[key]


def kernel(feat, src, dst, beta):
    cfg = make_cfg()
    nc = _get_nc(cfg)
    in_maps = prepare_inputs(feat, src, dst, beta, cfg)
    res = run_bass_kernel_spmd(nc, in_maps, core_ids=list(range(cfg.ncores)))
    return postprocess(res.results, cfg)



# revision 12
# speedup vs baseline: 1.2877x; 1.2368x over previous
"""AGNNConv on 8 Trainium2 NeuronCores — dense matmul formulation.

The per-edge attention weight exp(beta * cos(src, dst)) depends only on the
(src, dst) node pair, so the whole message passing collapses to dense algebra:

    G = norm^T norm                  (Gram matrix of L2-normalized features)
    H = C  *  exp(beta * G)          (C = dense dst-by-src edge-count matrix)
    num|den = H^T @ [feat | 1]  ;    out = num / den  rowwise

The count matrix C (dense, from the edge list), the L2-normalized transposed
features, and the [feat | 1] right-hand side are prepared on the host — all
O(N*D) or index work.  The device does the O(N^2 * D) dense work: for each
[128 src x 1024 dst] group, Gram matmuls (PE) -> exp (ACT) -> * C (DVE) ->
accumulating matmuls against [feat|1] (PE), then a rowwise divide.

Sharding: destination nodes are split across the 8 cores; each core computes
its npad/8 output rows end-to-end.  No collectives needed.

Engine balance (HW-measured): the kernel is paced by the C-matrix DMA
(~108us) with ACT/DVE/PE just below it.  ~24% of the exps run as a
Schraudolph bit-trick (a*x+b into int16, bitcast bf16) on the DVE to
unload the ScalarE, whose zero-depth exec queue costs ~266ns dispatch per
activation.  The all-padding source chunk is skipped (trim_pad), and the
output ships as bf16.  Rejected by measurement: fp8 matmuls in any role
(DoubleRow needs K=256; fp8 weights break the 2e-2 gate), uint8 C
transport (1-byte operands run ~3x slower on DVE), GpSimd offload
(tensor ops ~2.3us, casts ~3.9us per tile), and software pipelining lag
(the 4-deep OOO wait queue already covers the chain latency).
"""

import sys
import types

import numpy as np

try:
    from concourse import bacc, mybir, tile
    from concourse.bass_utils import run_bass_kernel_spmd
except ImportError:  # harness container may not have the repo on sys.path
    for _p in ("/opt/trn_rl_repo", "/root/.axon_site/_ro/trn_rl_repo"):
        if _p not in sys.path:
            sys.path.append(_p)
    from concourse import bacc, mybir, tile
    from concourse.bass_utils import run_bass_kernel_spmd

import ml_dtypes

F32 = mybir.dt.float32
BF16 = mybir.dt.bfloat16
FP8 = mybir.dt.float8e4
AF = mybir.ActivationFunctionType
ALU = mybir.AluOpType
DR = mybir.MatmulPerfMode.DoubleRow

D = 128  # feature dim
GFREE = 1024  # uniform group free size (gm * ck)


def _spread(n, count):
    """Pick `count` of n slots, evenly interleaved."""
    if count <= 0:
        return set()
    f = count / n
    return {g for g in range(n) if int((g + 1) * f) > int(g * f)}


def make_cfg(n_nodes=10000, npad=10240, ncores=8,
             lag=0, cb_pair=True, jfin=True, fp8_gram=False,
             fp8_plain=False, n_gpx=0, trim_pad=True,
             n_dve_exp=24, n_gp_mult=0, n_dve_u8=0, out_bf16=True,
             cast_dma=False, cbw=4):
    c = types.SimpleNamespace()
    c.cast_dma = cast_dma      # ship C as u8, SWDGE cast-DMA expands to bf16
    c.n_nodes = n_nodes
    c.npad = npad              # padded node count (multiple of 128*ncores)
    c.ncores = ncores
    c.fp8_gram = fp8_gram      # Gram matmuls in fp8 DoubleRow (2x PE rate)
    c.fp8_plain = fp8_plain    # Gram operands fp8 (same PE rate, half DMA)
    c.npc = npad // ncores     # dst columns per core
    c.mch = npad // 128        # source-node chunks (contraction dim)
    c.tt = c.npc // 128        # output row-tiles per core
    c.lag = lag                # groups of G->B software pipelining
    c.cb_pair = cb_pair        # 2-group C DMA batching
    c.cbw = cbw if cb_pair else 1  # groups per C DMA batch
    c.jfin = jfin              # finalize/ship output per j-chunk
    # j-chunks of dst columns: prefer 512 wide, remainder in one chunk
    c.jchunks = []             # (joff, ck, gm, ngroups_j)
    off = 0
    while off < c.npc:
        ck = min(512, c.npc - off)
        assert ck % 128 == 0 and GFREE % ck == 0
        gm = GFREE // ck
        assert c.mch % gm == 0
        c.jchunks.append((off, ck, gm, c.mch * ck // GFREE))
        off += ck
    c.ngroups = c.mch * c.npc // GFREE
    # Per-group engine/dtype assignment, each class evenly interleaved.
    # exp: ACT activation vs DVE Schraudolph bit-trick.
    # mult: DVE tensor_tensor vs GpSimd.
    # C transport: bf16 (2x DVE mult) vs uint8 (half DMA, 1x mult; free
    # on GpSimd, whose cost is dtype-independent).
    c.n_dve_exp, c.n_gp_mult, c.n_dve_u8 = n_dve_exp, n_gp_mult, n_dve_u8
    c.n_gpx = n_gpx
    c.out_bf16 = out_bf16
    c.trim_pad = trim_pad
    # last fully-padded source chunk (nodes >= n_nodes): skip its work
    c.pad_chunk = c.mch - 1 if trim_pad and n_nodes <= (c.mch - 1) * 128 else -1
    c.dve_exp_groups = _spread(c.ngroups, n_dve_exp)
    c.gp_groups = _spread(c.ngroups, n_gp_mult)
    c.gpx_groups = _spread(c.ngroups, n_gpx) - c.gp_groups
    dve_mult = [g for g in range(c.ngroups) if g not in c.gp_groups]
    u8_dve = {dve_mult[i] for i in sorted(
        {int(j * len(dve_mult) / max(n_dve_u8, 1)) for j in range(n_dve_u8)}
    )} if n_dve_u8 else set()
    u8set = c.gp_groups | u8_dve | c.gpx_groups
    c.u8_groups = sorted(u8set)
    c.bf_groups = [g for g in range(c.ngroups) if g not in u8set]
    return c


def build(cfg):
    """Build the per-core SPMD graph (identical on all cores; data differs)."""
    nc = bacc.Bacc(
        "TRN2", target_bir_lowering=False, debug=False, num_devices=cfg.ncores
    )
    D1 = D + 1
    gdt = FP8 if cfg.fp8_plain else BF16
    if cfg.fp8_gram:
        # features split into two 64-row K-tiles for DoubleRow fp8 matmul
        ntd = nc.dram_tensor("normT", [64, 2, cfg.npad], FP8, kind="ExternalInput")
        nmd = nc.dram_tensor("normTmy", [64, 2, cfg.npc], FP8, kind="ExternalInput")
    else:
        ntd = nc.dram_tensor("normT", [128, cfg.npad], gdt, kind="ExternalInput")
        nmd = nc.dram_tensor("normTmy", [128, cfg.npc], gdt, kind="ExternalInput")
    fqd = nc.dram_tensor("featq", [128, cfg.mch * D1], BF16, kind="ExternalInput")
    n16, n8 = len(cfg.bf_groups), len(cfg.u8_groups)
    ctd = nc.dram_tensor(
        "ct", [128, n16 * GFREE],
        mybir.dt.uint8 if cfg.cast_dma else BF16, kind="ExternalInput"
    )
    ct8d = (
        nc.dram_tensor("ct8", [128, n8 * GFREE], mybir.dt.uint8,
                       kind="ExternalInput")
        if n8 else None
    )
    odt = BF16 if cfg.out_bf16 else F32
    outd = nc.dram_tensor("out", [128, cfg.tt, D], odt, kind="ExternalOutput")

    with tile.TileContext(nc) as tc:
        with (
            tc.tile_pool(name="const", bufs=1) as constp,
            tc.tile_pool(name="big", bufs=1) as bigp,
            tc.tile_pool(name="cb", bufs=5) as cbp,
            tc.tile_pool(name="cbx", bufs=6) as cbxp,
            tc.tile_pool(name="eg", bufs=8) as egp,
            tc.tile_pool(name="ht", bufs=8) as htp,
            tc.tile_pool(name="pg", bufs=3, space="PSUM") as pgp,
            tc.tile_pool(name="po", bufs=2, space="PSUM") as pop,
        ):
            if cfg.fp8_gram:
                normT = bigp.tile([64, 2, cfg.npad], FP8)
                normTmy = bigp.tile([64, 2, cfg.npc], FP8)
            else:
                normT = bigp.tile([128, cfg.npad], gdt)
                normTmy = bigp.tile([128, cfg.npc], gdt)
            featq = bigp.tile([128, cfg.mch * D1], BF16)
            outacc = bigp.tile([128, cfg.tt, D1], F32)
            final = bigp.tile([128, cfg.npc], BF16 if cfg.out_bf16 else F32)

            def nt_slice(a, b):
                return (normT[:, :, a:b], ntd[:, :, a:b]) if cfg.fp8_gram else (
                    normT[:, a:b], ntd[:, a:b])

            # group-0 blockers first: first normT slice, first nmy j-slice
            nc.sync.dma_start(*nt_slice(0, 256))
            if cfg.fp8_gram:
                nc.sync.dma_start(normTmy[:], nmd[:])
            else:
                nc.sync.dma_start(normTmy[:, 0:512], nmd[:, 0:512])
                for a, b in ((512, 1024), (1024, cfg.npc)):
                    if b > a:
                        nc.sync.dma_start(normTmy[:, a:b], nmd[:, a:b])
            nsplit = 8
            stepn = max(128, (cfg.npad // nsplit) // 128 * 128)
            stepq = max(D1, (cfg.mch * D1 // nsplit) // D1 * D1)
            qoffs = list(range(0, cfg.mch * D1, stepq))
            noffs = list(range(0, cfg.npad, stepn))
            cbw = cfg.cbw
            pos16 = {g: i for i, g in enumerate(cfg.bf_groups)}
            pos8 = {g: i for i, g in enumerate(cfg.u8_groups)}
            cb_tiles = {"16": {}, "8": {}}
            exp8 = {}

            def fetch_cb_pair(mod, pix):
                # one DMA covering cbw consecutive same-modality groups
                if mod == "16":
                    total, dram, dt, tg = len(cfg.bf_groups) * GFREE, ctd, BF16, "cb"
                else:
                    total, dram, dt, tg = (
                        len(cfg.u8_groups) * GFREE, ct8d, mybir.dt.uint8, "cb8"
                    )
                lo = pix * cbw * GFREE
                hi = min((pix * cbw + cbw) * GFREE, total)
                cbt = cbp.tile([128, cbw * GFREE], dt, tag=tg, name=f"cb{mod}")
                if mod == "16" and cfg.cast_dma:
                    # C travels as u8 in HBM; the SWDGE datapath widens to
                    # bf16 on the SBUF write side (halves HBM-side traffic)
                    nc.gpsimd.dma_start(cbt[:, 0 : hi - lo], dram[:, lo:hi])
                else:
                    nc.sync.dma_start(cbt[:, 0 : hi - lo], dram[:, lo:hi])
                cb_tiles[mod][pix] = cbt
                if mod == "8":
                    # u8 -> bf16 expansion on the (otherwise idle) GpSimd,
                    # off the critical path: depends only on the C DMA
                    for h in range((hi - lo) // GFREE):
                        g8 = cfg.u8_groups[pix * cbw + h]
                        if g8 in cfg.gpx_groups:
                            xt = cbxp.tile([128, GFREE], BF16, tag="cbx",
                                           name="cbx")
                            nc.gpsimd.tensor_copy(
                                xt[:], cbt[:, h * GFREE : (h + 1) * GFREE]
                            )
                            exp8[g8] = xt

            def get_cb(g):
                mod = "8" if g in pos8 else "16"
                pos = pos8[g] if mod == "8" else pos16[g]
                pix = pos // cbw
                if pix not in cb_tiles[mod]:
                    fetch_cb_pair(mod, pix)
                if mod == "8" and (pix + 1) * cbw < len(cfg.u8_groups) and (
                    pix + 1
                ) not in cb_tiles[mod]:
                    # lookahead so GpSimd expansion leads the consumer
                    fetch_cb_pair(mod, pix + 1)
                t = cb_tiles[mod][pix]
                if pos % cbw == cbw - 1 or g == cfg.ngroups - 1:
                    cb_tiles[mod].pop(pix)
                if g in cfg.gpx_groups:
                    return exp8.pop(g)[:]
                return t[:, (pos % cbw) * GFREE : (pos % cbw + 1) * GFREE]

            nc.sync.dma_start(featq[:, 0 : 8 * D1], fqd[:, 0 : 8 * D1])
            for ix in range(max(len(qoffs), len(noffs))):
                if ix < 3:  # stream C from t=0
                    if ix * cbw < len(cfg.bf_groups):
                        fetch_cb_pair("16", ix)
                    if ix * cbw < len(cfg.u8_groups):
                        fetch_cb_pair("8", ix)
                if ix < len(noffs):
                    a = max(noffs[ix], 256 if ix == 0 else 0)
                    b = min(noffs[ix] + stepn, cfg.npad)
                    if b > a:
                        nc.sync.dma_start(*nt_slice(a, b))
                if ix < len(qoffs):
                    a = qoffs[ix] + (8 * D1 if ix == 0 else 0)
                    b = min(qoffs[ix] + stepq, cfg.mch * D1)
                    if b > a:
                        nc.sync.dma_start(featq[:, a:b], fqd[:, a:b])

            dmax = constp.tile([128, cfg.tt], F32)
            rden = constp.tile([128, cfg.tt], F32)

            # ---- main loop: uniform [128, GFREE] groups, software-
            # pipelined so a group's B-matmuls trail its G-matmuls by
            # cfg.lag groups (hides the psum->exp->mult latency on PE) ----
            groups = []  # flat (joff, ck, gm, ns, gj, ngj, jix)
            for jix, (joff, ck, gm, ngj) in enumerate(cfg.jchunks):
                for gj in range(ngj):
                    groups.append((joff, ck, gm, ck // 128, gj, ngj, jix))
            po_by_j = {}
            ht_by_g = {}

            def emit_front(gidx):
                joff, ck, gm, ns, gj, ngj, jix = groups[gidx]
                if jix not in po_by_j:
                    po_by_j[jix] = [
                        pop.tile([128, 2 * D1], F32, tag="po", name=f"po{jix}_{u}")
                        for u in range((ns + 1) // 2)
                    ]
                pg = pgp.tile([128, GFREE], F32, tag="pg", name="pg")
                kuse = gm - (1 if gj * gm + gm - 1 == cfg.pad_chunk else 0)
                fr = kuse * ck  # active free size (pad chunk trimmed)
                for k in range(kuse):
                    i = gj * gm + k
                    if cfg.fp8_gram:
                        nc.tensor.matmul(
                            pg[:, k * ck : (k + 1) * ck],
                            normT[:, :, i * 128 : (i + 1) * 128],
                            normTmy[:, :, joff : joff + ck],
                            start=True, stop=True, perf_mode=DR,
                        )
                    else:
                        nc.tensor.matmul(
                            pg[:, k * ck : (k + 1) * ck],
                            normT[:, i * 128 : (i + 1) * 128],
                            normTmy[:, joff : joff + ck],
                            start=True, stop=True,
                        )
                cb = get_cb(gidx)
                mult_eng = (
                    nc.gpsimd if gidx in cfg.gp_groups else nc.vector
                )
                ht = htp.tile([128, GFREE], BF16, tag="ht", name="ht")
                if gidx in cfg.dve_exp_groups:
                    # exp(x) ~= bf16_bits(round(184.665*x + 16250.4)):
                    # Schraudolph bit-trick on DVE, offloading ScalarE
                    si = egp.tile(
                        [128, GFREE], mybir.dt.int16, tag="eg", name="si"
                    )
                    nc.vector.tensor_scalar(
                        out=si[:, 0:fr], in0=pg[:, 0:fr], scalar1=184.664965,
                        scalar2=16250.4, op0=ALU.mult, op1=ALU.add,
                    )
                    mult_eng.tensor_tensor(
                        ht[:, 0:fr], si[:, 0:fr].bitcast(BF16), cb[:, 0:fr],
                        op=ALU.mult,
                    )
                else:
                    eg = egp.tile([128, GFREE], BF16, tag="eg", name="eg")
                    nc.scalar.activation(eg[:, 0:fr], pg[:, 0:fr], AF.Exp)
                    mult_eng.tensor_tensor(
                        ht[:, 0:fr], eg[:, 0:fr], cb[:, 0:fr], op=ALU.mult
                    )
                ht_by_g[gidx] = ht

            def emit_back(gidx):
                joff, ck, gm, ns, gj, ngj, jix = groups[gidx]
                ht = ht_by_g.pop(gidx)
                po = po_by_j[jix]
                kuse = gm - (1 if gj * gm + gm - 1 == cfg.pad_chunk else 0)
                for k in range(kuse):
                    i = gj * gm + k
                    for s in range(ns):
                        nc.tensor.matmul(
                            po[s // 2][:, (s % 2) * D1 : (s % 2 + 1) * D1],
                            ht[:, k * ck + s * 128 : k * ck + (s + 1) * 128],
                            featq[:, i * D1 : (i + 1) * D1],
                            # start zeroes the whole 2KB PSUM bank, so only
                            # the first region of each packed pair sets it
                            start=(gj == 0 and k == 0 and s % 2 == 0),
                            stop=(gj == ngj - 1 and k == kuse - 1),
                            skip_group_check=True,
                        )
                if gj == ngj - 1:  # last group of this j-chunk: drain po,
                    t0 = joff // 128   # divide and ship this slice out now
                    for s in range(ns):
                        nc.vector.tensor_copy(
                            outacc[:, t0 + s, :],
                            po[s // 2][:, (s % 2) * D1 : (s % 2 + 1) * D1],
                        )
                    if cfg.jfin:
                        finalize_j(joff, ns)

            def finalize_j(joff, ns):
                    t0 = joff // 128
                    nc.vector.tensor_scalar(
                        out=dmax[:, t0 : t0 + ns],
                        in0=outacc[:, t0 : t0 + ns, D : D + 1],
                        scalar1=1e-30, scalar2=None, op0=ALU.max,
                    )
                    nc.vector.reciprocal(
                        rden[:, t0 : t0 + ns], dmax[:, t0 : t0 + ns]
                    )
                    for s in range(ns):
                        t = t0 + s
                        nc.vector.tensor_scalar(
                            out=final[:, t * D : (t + 1) * D],
                            in0=outacc[:, t, 0:D],
                            scalar1=rden[:, t : t + 1], scalar2=None,
                            op0=ALU.mult,
                        )
                    nc.sync.dma_start(
                        outd[:, t0 : t0 + ns, :],
                        final[:, t0 * D : (t0 + ns) * D].rearrange(
                            "p (t d) -> p t d", d=D
                        ),
                    )

            for g in range(cfg.ngroups + cfg.lag):
                if g < cfg.ngroups:
                    emit_front(g)
                if g >= cfg.lag:
                    emit_back(g - cfg.lag)
            if not cfg.jfin:
                for joff, ck, gm, ngj in cfg.jchunks:
                    finalize_j(joff, ck // 128)

    nc.compile()
    return nc


def prepare_inputs(feat, src, dst, beta, cfg):
    feat = np.ascontiguousarray(np.asarray(feat), dtype=np.float32)
    src = np.asarray(src).astype(np.int64)
    dst = np.asarray(dst).astype(np.int64)
    beta = np.asarray(beta, dtype=np.float32).reshape(-1)
    D1 = D + 1

    featp = np.zeros((cfg.npad, D), np.float32)
    featp[: cfg.n_nodes] = feat
    rn = 1.0 / np.maximum(np.linalg.norm(featp, axis=1, keepdims=True), 1e-12)
    normp = featp * rn
    if cfg.fp8_gram:
        # [64, 2, npad]: feature rows split into two 64-row K-tiles
        normT = np.ascontiguousarray(
            normp.T.reshape(2, 64, cfg.npad).transpose(1, 0, 2)
            .astype(ml_dtypes.float8_e4m3fn)
        )
    elif cfg.fp8_plain:
        normT = np.ascontiguousarray(normp.T.astype(ml_dtypes.float8_e4m3fn))
    else:
        normT = np.ascontiguousarray(normp.T.astype(ml_dtypes.bfloat16))

    # featq: [128, mch*(D+1)] bf16; block i col D holds the bias 1.0
    fq = np.ones((128, cfg.mch, D1), dtype=ml_dtypes.bfloat16)
    fq[:, :, :D] = (
        featp.astype(ml_dtypes.bfloat16).reshape(cfg.mch, 128, D).transpose(1, 0, 2)
    )
    fq = np.ascontiguousarray(fq.reshape(128, cfg.mch * D1))

    in_maps = []
    for c in range(cfg.ncores):
        lo = c * cfg.npc
        nmyT = (beta[0] * normp[lo : lo + cfg.npc]).T  # [128, npc]
        if cfg.fp8_gram:
            nmy = np.ascontiguousarray(
                nmyT.reshape(2, 64, cfg.npc).transpose(1, 0, 2)
                .astype(ml_dtypes.float8_e4m3fn)
            )
        elif cfg.fp8_plain:
            nmy = np.ascontiguousarray(nmyT.astype(ml_dtypes.float8_e4m3fn))
        else:
            nmy = np.ascontiguousarray(nmyT.astype(ml_dtypes.bfloat16))
        m = (dst >= lo) & (dst < lo + cfg.npc)
        s_c = src[m]
        d_c = dst[m] - lo
        cnt = np.bincount(
            s_c * cfg.npc + d_c, minlength=cfg.npad * cfg.npc
        ).reshape(cfg.npad, cfg.npc)
        # group-major C layout: per j-chunk, per group: [128, gm*ck]
        blocks = []
        for joff, ck, gm, ngj in cfg.jchunks:
            blk = cnt[:, joff : joff + ck].reshape(ngj, gm, 128, ck)
            blocks.append(blk.transpose(2, 0, 1, 3).reshape(128, ngj * gm * ck))
        ctall = np.concatenate(blocks, axis=1)
        gb = ctall.reshape(128, cfg.ngroups, GFREE)
        if cfg.cast_dma:
            assert ctall.max() <= 255
            ct = np.ascontiguousarray(
                gb[:, cfg.bf_groups, :].reshape(128, -1).astype(np.uint8)
            )
        else:
            ct = np.ascontiguousarray(
                gb[:, cfg.bf_groups, :].reshape(128, -1).astype(ml_dtypes.bfloat16)
            )
        im = {"normT": normT, "normTmy": nmy, "featq": fq, "ct": ct}
        if cfg.u8_groups:
            assert ctall.max() <= 255
            im["ct8"] = np.ascontiguousarray(
                gb[:, cfg.u8_groups, :].reshape(128, -1).astype(np.uint8)
            )
        in_maps.append(im)
    return in_maps


def postprocess(results, cfg):
    parts = []
    for c in range(cfg.ncores):
        o = np.asarray(results[c]["out"], np.float32)  # [128, tt, D]
        parts.append(o.transpose(1, 0, 2).reshape(cfg.npc, D))
    return np.concatenate(parts, axis=0)[: cfg.n_nodes]


_CACHE = {}


def _get_nc(cfg):
    key = (cfg.npad, cfg.ncores, cfg.n_dve_exp, cfg.n_gp_mult, cfg.lag,
           cfg.cb_pair, cfg.jfin, tuple(cfg.u8_groups), cfg.fp8_gram,
           cfg.out_bf16, cfg.fp8_plain, cfg.n_gpx, cfg.pad_chunk,
           cfg.cast_dma, cfg.cbw)
    if key not in _CACHE:
        _CACHE[key] = build(cfg)
    return _CACHE[key]


def kernel(feat, src, dst, beta):
    cfg = make_cfg()
    nc = _get_nc(cfg)
    in_maps = prepare_inputs(feat, src, dst, beta, cfg)
    res = run_bass_kernel_spmd(nc, in_maps, core_ids=list(range(cfg.ncores)))
    return postprocess(res.results, cfg)



# revision 18
# speedup vs baseline: 1.3241x; 1.0282x over previous
"""AGNNConv on 8 Trainium2 NeuronCores — dense matmul formulation.

The per-edge attention weight exp(beta * cos(src, dst)) depends only on the
(src, dst) node pair, so the whole message passing collapses to dense algebra:

    G = norm^T norm                  (Gram matrix of L2-normalized features)
    H = C  *  exp(beta * G)          (C = dense dst-by-src edge-count matrix)
    num|den = H^T @ [feat | 1]  ;    out = num / den  rowwise

The count matrix C (dense, from the edge list), the L2-normalized transposed
features, and the [feat | 1] right-hand side are prepared on the host — all
O(N*D) or index work.  The device does the O(N^2 * D) dense work: for each
[128 src x 1024 dst] group, Gram matmuls (PE) -> exp (ACT) -> * C (DVE) ->
accumulating matmuls against [feat|1] (PE), then a rowwise divide.

Sharding: destination nodes are split across the 8 cores; each core computes
its npad/8 output rows end-to-end.  No collectives needed.

Engine balance (HW-measured): the kernel is paced by the C-matrix DMA
(~108us) with ACT/DVE/PE just below it.  ~24% of the exps run as a
Schraudolph bit-trick (a*x+b into int16, bitcast bf16) on the DVE to
unload the ScalarE, whose zero-depth exec queue costs ~266ns dispatch per
activation.  The all-padding source chunk is skipped (trim_pad), and the
output ships as bf16.  Rejected by measurement: fp8 matmuls in any role
(DoubleRow needs K=256; fp8 weights break the 2e-2 gate), uint8 C
transport (1-byte operands run ~3x slower on DVE), GpSimd offload
(tensor ops ~2.3us, casts ~3.9us per tile), and software pipelining lag
(the 4-deep OOO wait queue already covers the chain latency).
"""

import sys
import types

import numpy as np

try:
    from concourse import bacc, mybir, tile
    from concourse.bass_utils import run_bass_kernel_spmd
except ImportError:  # harness container may not have the repo on sys.path
    for _p in ("/opt/trn_rl_repo", "/root/.axon_site/_ro/trn_rl_repo"):
        if _p not in sys.path:
            sys.path.append(_p)
    from concourse import bacc, mybir, tile
    from concourse.bass_utils import run_bass_kernel_spmd

import ml_dtypes

F32 = mybir.dt.float32
BF16 = mybir.dt.bfloat16
FP8 = mybir.dt.float8e4
AF = mybir.ActivationFunctionType
ALU = mybir.AluOpType
DR = mybir.MatmulPerfMode.DoubleRow

D = 128  # feature dim
GFREE = 1024  # uniform group free size (gm * ck)


def _spread(n, count):
    """Pick `count` of n slots, evenly interleaved."""
    if count <= 0:
        return set()
    f = count / n
    return {g for g in range(n) if int((g + 1) * f) > int(g * f)}


def make_cfg(n_nodes=10000, npad=10240, ncores=8,
             lag=0, cb_pair=True, jfin=True, fp8_gram=False,
             fp8_plain=False, n_gpx=0, trim_pad=True,
             n_dve_exp=24, n_gp_mult=0, n_dve_u8=0, out_bf16=True,
             cast_dma=False, cbw=4, pair_mult=True):
    c = types.SimpleNamespace()
    c.cast_dma = cast_dma      # ship C as u8, SWDGE cast-DMA expands to bf16
    c.n_nodes = n_nodes
    c.npad = npad              # padded node count (multiple of 128*ncores)
    c.ncores = ncores
    c.fp8_gram = fp8_gram      # Gram matmuls in fp8 DoubleRow (2x PE rate)
    c.fp8_plain = fp8_plain    # Gram operands fp8 (same PE rate, half DMA)
    c.npc = npad // ncores     # dst columns per core
    c.mch = npad // 128        # source-node chunks (contraction dim)
    c.tt = c.npc // 128        # output row-tiles per core
    c.lag = lag                # groups of G->B software pipelining
    c.cb_pair = cb_pair        # 2-group C DMA batching
    c.cbw = cbw if cb_pair else 1  # groups per C DMA batch
    c.jfin = jfin              # finalize/ship output per j-chunk
    # j-chunks of dst columns: prefer 512 wide, remainder in one chunk
    c.jchunks = []             # (joff, ck, gm, ngroups_j)
    off = 0
    while off < c.npc:
        ck = min(512, c.npc - off)
        assert ck % 128 == 0 and GFREE % ck == 0
        gm = GFREE // ck
        assert c.mch % gm == 0
        c.jchunks.append((off, ck, gm, c.mch * ck // GFREE))
        off += ck
    c.ngroups = c.mch * c.npc // GFREE
    # Per-group engine/dtype assignment, each class evenly interleaved.
    # exp: ACT activation vs DVE Schraudolph bit-trick.
    # mult: DVE tensor_tensor vs GpSimd.
    # C transport: bf16 (2x DVE mult) vs uint8 (half DMA, 1x mult; free
    # on GpSimd, whose cost is dtype-independent).
    c.n_dve_exp, c.n_gp_mult, c.n_dve_u8 = n_dve_exp, n_gp_mult, n_dve_u8
    c.n_gpx = n_gpx
    c.out_bf16 = out_bf16
    c.trim_pad = trim_pad
    # last fully-padded source chunk (nodes >= n_nodes): skip its work
    c.pad_chunk = c.mch - 1 if trim_pad and n_nodes <= (c.mch - 1) * 128 else -1
    c.dve_exp_groups = _spread(c.ngroups, n_dve_exp)
    c.gp_groups = _spread(c.ngroups, n_gp_mult)
    c.gpx_groups = _spread(c.ngroups, n_gpx) - c.gp_groups
    dve_mult = [g for g in range(c.ngroups) if g not in c.gp_groups]
    u8_dve = {dve_mult[i] for i in sorted(
        {int(j * len(dve_mult) / max(n_dve_u8, 1)) for j in range(n_dve_u8)}
    )} if n_dve_u8 else set()
    u8set = c.gp_groups | u8_dve | c.gpx_groups
    c.u8_groups = sorted(u8set)
    c.bf_groups = [g for g in range(c.ngroups) if g not in u8set]
    # pair-mult: one DVE tensor_tensor covers two consecutive groups
    # (1127ns per 2048-wide op vs 2x679 per 1024) — requires all-bf16 C,
    # even group counts per j-chunk, pairs aligned with the C batches
    c.pair_mult = (pair_mult and not c.u8_groups and c.cbw % 2 == 0
                   and all(ngj % 2 == 0 for _, _, _, ngj in c.jchunks))
    return c


def build(cfg):
    """Build the per-core SPMD graph (identical on all cores; data differs)."""
    nc = bacc.Bacc(
        "TRN2", target_bir_lowering=False, debug=False, num_devices=cfg.ncores
    )
    D1 = D + 1
    gdt = FP8 if cfg.fp8_plain else BF16
    if cfg.fp8_gram:
        # features split into two 64-row K-tiles for DoubleRow fp8 matmul
        ntd = nc.dram_tensor("normT", [64, 2, cfg.npad], FP8, kind="ExternalInput")
        nmd = nc.dram_tensor("normTmy", [64, 2, cfg.npc], FP8, kind="ExternalInput")
    else:
        ntd = nc.dram_tensor("normT", [128, cfg.npad], gdt, kind="ExternalInput")
        nmd = nc.dram_tensor("normTmy", [128, cfg.npc], gdt, kind="ExternalInput")
    fqd = nc.dram_tensor("featq", [128, cfg.mch * D1], BF16, kind="ExternalInput")
    n16, n8 = len(cfg.bf_groups), len(cfg.u8_groups)
    ctd = nc.dram_tensor(
        "ct", [128, n16 * GFREE],
        mybir.dt.uint8 if cfg.cast_dma else BF16, kind="ExternalInput"
    )
    ct8d = (
        nc.dram_tensor("ct8", [128, n8 * GFREE], mybir.dt.uint8,
                       kind="ExternalInput")
        if n8 else None
    )
    odt = BF16 if cfg.out_bf16 else F32
    outd = nc.dram_tensor("out", [128, cfg.tt, D], odt, kind="ExternalOutput")

    with tile.TileContext(nc) as tc:
        with (
            tc.tile_pool(name="const", bufs=1) as constp,
            tc.tile_pool(name="big", bufs=1) as bigp,
            tc.tile_pool(name="cb", bufs=5) as cbp,
            tc.tile_pool(name="cbx", bufs=6) as cbxp,
            tc.tile_pool(name="eg", bufs=8) as egp,
            tc.tile_pool(name="ht", bufs=8) as htp,
            tc.tile_pool(name="pg", bufs=3, space="PSUM") as pgp,
            tc.tile_pool(name="po", bufs=2, space="PSUM") as pop,
        ):
            if cfg.fp8_gram:
                normT = bigp.tile([64, 2, cfg.npad], FP8)
                normTmy = bigp.tile([64, 2, cfg.npc], FP8)
            else:
                normT = bigp.tile([128, cfg.npad], gdt)
                normTmy = bigp.tile([128, cfg.npc], gdt)
            featq = bigp.tile([128, cfg.mch * D1], BF16)
            outacc = bigp.tile([128, cfg.tt, D1], F32)
            final = bigp.tile([128, cfg.npc], BF16 if cfg.out_bf16 else F32)

            def nt_slice(a, b):
                return (normT[:, :, a:b], ntd[:, :, a:b]) if cfg.fp8_gram else (
                    normT[:, a:b], ntd[:, a:b])

            # group-0 blockers first: first normT slice, first nmy j-slice
            nc.sync.dma_start(*nt_slice(0, 256))
            if cfg.fp8_gram:
                nc.sync.dma_start(normTmy[:], nmd[:])
            else:
                nc.sync.dma_start(normTmy[:, 0:512], nmd[:, 0:512])
                for a, b in ((512, 1024), (1024, cfg.npc)):
                    if b > a:
                        nc.sync.dma_start(normTmy[:, a:b], nmd[:, a:b])
            nsplit = 8
            stepn = max(128, (cfg.npad // nsplit) // 128 * 128)
            stepq = max(D1, (cfg.mch * D1 // nsplit) // D1 * D1)
            qoffs = list(range(0, cfg.mch * D1, stepq))
            noffs = list(range(0, cfg.npad, stepn))
            cbw = cfg.cbw
            pos16 = {g: i for i, g in enumerate(cfg.bf_groups)}
            pos8 = {g: i for i, g in enumerate(cfg.u8_groups)}
            cb_tiles = {"16": {}, "8": {}}
            exp8 = {}

            def fetch_cb_pair(mod, pix):
                # one DMA covering cbw consecutive same-modality groups
                if mod == "16":
                    total, dram, dt, tg = len(cfg.bf_groups) * GFREE, ctd, BF16, "cb"
                else:
                    total, dram, dt, tg = (
                        len(cfg.u8_groups) * GFREE, ct8d, mybir.dt.uint8, "cb8"
                    )
                lo = pix * cbw * GFREE
                hi = min((pix * cbw + cbw) * GFREE, total)
                cbt = cbp.tile([128, cbw * GFREE], dt, tag=tg, name=f"cb{mod}")
                if mod == "16" and cfg.cast_dma:
                    # C travels as u8 in HBM; the SWDGE datapath widens to
                    # bf16 on the SBUF write side (halves HBM-side traffic)
                    nc.gpsimd.dma_start(cbt[:, 0 : hi - lo], dram[:, lo:hi])
                else:
                    nc.sync.dma_start(cbt[:, 0 : hi - lo], dram[:, lo:hi])
                cb_tiles[mod][pix] = cbt
                if mod == "8":
                    # u8 -> bf16 expansion on the (otherwise idle) GpSimd,
                    # off the critical path: depends only on the C DMA
                    for h in range((hi - lo) // GFREE):
                        g8 = cfg.u8_groups[pix * cbw + h]
                        if g8 in cfg.gpx_groups:
                            xt = cbxp.tile([128, GFREE], BF16, tag="cbx",
                                           name="cbx")
                            nc.gpsimd.tensor_copy(
                                xt[:], cbt[:, h * GFREE : (h + 1) * GFREE]
                            )
                            exp8[g8] = xt

            def get_cb(g):
                mod = "8" if g in pos8 else "16"
                pos = pos8[g] if mod == "8" else pos16[g]
                pix = pos // cbw
                if pix not in cb_tiles[mod]:
                    fetch_cb_pair(mod, pix)
                if mod == "8" and (pix + 1) * cbw < len(cfg.u8_groups) and (
                    pix + 1
                ) not in cb_tiles[mod]:
                    # lookahead so GpSimd expansion leads the consumer
                    fetch_cb_pair(mod, pix + 1)
                t = cb_tiles[mod][pix]
                if pos % cbw == cbw - 1 or g == cfg.ngroups - 1:
                    cb_tiles[mod].pop(pix)
                if g in cfg.gpx_groups:
                    return exp8.pop(g)[:]
                return t[:, (pos % cbw) * GFREE : (pos % cbw + 1) * GFREE]

            def get_cb2(p):
                # contiguous 2-group C slice for a pair (all-bf16 mode)
                pix = p // cbw
                if pix not in cb_tiles["16"]:
                    fetch_cb_pair("16", pix)
                t = cb_tiles["16"][pix]
                lo = (p % cbw) * GFREE
                if p % cbw == cbw - 2 or p + 2 >= cfg.ngroups:
                    cb_tiles["16"].pop(pix)
                return t[:, lo : lo + 2 * GFREE]

            nc.sync.dma_start(featq[:, 0 : 8 * D1], fqd[:, 0 : 8 * D1])
            for ix in range(max(len(qoffs), len(noffs))):
                if ix < 3:  # stream C from t=0
                    if ix * cbw < len(cfg.bf_groups):
                        fetch_cb_pair("16", ix)
                    if ix * cbw < len(cfg.u8_groups):
                        fetch_cb_pair("8", ix)
                if ix < len(noffs):
                    a = max(noffs[ix], 256 if ix == 0 else 0)
                    b = min(noffs[ix] + stepn, cfg.npad)
                    if b > a:
                        nc.sync.dma_start(*nt_slice(a, b))
                if ix < len(qoffs):
                    a = qoffs[ix] + (8 * D1 if ix == 0 else 0)
                    b = min(qoffs[ix] + stepq, cfg.mch * D1)
                    if b > a:
                        nc.sync.dma_start(featq[:, a:b], fqd[:, a:b])

            dmax = constp.tile([128, cfg.tt], F32)
            rden = constp.tile([128, cfg.tt], F32)

            # ---- main loop: uniform [128, GFREE] groups, software-
            # pipelined so a group's B-matmuls trail its G-matmuls by
            # cfg.lag groups (hides the psum->exp->mult latency on PE) ----
            groups = []  # flat (joff, ck, gm, ns, gj, ngj, jix)
            for jix, (joff, ck, gm, ngj) in enumerate(cfg.jchunks):
                for gj in range(ngj):
                    groups.append((joff, ck, gm, ck // 128, gj, ngj, jix))
            po_by_j = {}
            ht_by_g = {}

            def gram_exp(gidx, edst, eoff):
                # Gram matmuls + exp for one group; exp lands in
                # edst[:, eoff:eoff+fr] (bf16 bits either way: ACT writes
                # bf16, Schraudolph writes the int16 view of the same slice)
                joff, ck, gm, ns, gj, ngj, jix = groups[gidx]
                if jix not in po_by_j:
                    po_by_j[jix] = [
                        pop.tile([128, 2 * D1], F32, tag="po", name=f"po{jix}_{u}")
                        for u in range((ns + 1) // 2)
                    ]
                pg = pgp.tile([128, GFREE], F32, tag="pg", name="pg")
                kuse = gm - (1 if gj * gm + gm - 1 == cfg.pad_chunk else 0)
                fr = kuse * ck  # active free size (pad chunk trimmed)
                for k in range(kuse):
                    i = gj * gm + k
                    if cfg.fp8_gram:
                        nc.tensor.matmul(
                            pg[:, k * ck : (k + 1) * ck],
                            normT[:, :, i * 128 : (i + 1) * 128],
                            normTmy[:, :, joff : joff + ck],
                            start=True, stop=True, perf_mode=DR,
                        )
                    else:
                        nc.tensor.matmul(
                            pg[:, k * ck : (k + 1) * ck],
                            normT[:, i * 128 : (i + 1) * 128],
                            normTmy[:, joff : joff + ck],
                            start=True, stop=True,
                        )
                if gidx in cfg.dve_exp_groups:
                    # exp(x) ~= bf16_bits(round(184.665*x + 16250.4)):
                    # Schraudolph bit-trick on DVE, offloading ScalarE
                    nc.vector.tensor_scalar(
                        out=edst[:, eoff : eoff + fr].bitcast(mybir.dt.int16),
                        in0=pg[:, 0:fr], scalar1=184.664965,
                        scalar2=16250.4, op0=ALU.mult, op1=ALU.add,
                    )
                else:
                    nc.scalar.activation(
                        edst[:, eoff : eoff + fr], pg[:, 0:fr], AF.Exp
                    )
                return fr

            def emit_front_pair(p):
                # two groups share one eg tile and one wide DVE mult
                ep = egp.tile([128, 2 * GFREE], BF16, tag="eg", name="ep")
                fr0 = gram_exp(p, ep, 0)
                fr1 = gram_exp(p + 1, ep, GFREE)
                assert fr0 == GFREE
                frp = GFREE + fr1
                cb = get_cb2(p)
                ht = htp.tile([128, 2 * GFREE], BF16, tag="ht", name="ht")
                nc.vector.tensor_tensor(
                    ht[:, 0:frp], ep[:, 0:frp], cb[:, 0:frp], op=ALU.mult
                )
                ht_by_g[p] = (ht, 0)
                ht_by_g[p + 1] = (ht, GFREE)

            def emit_front(gidx):
                eg = egp.tile([128, GFREE], BF16, tag="eg", name="eg")
                fr = gram_exp(gidx, eg, 0)
                cb = get_cb(gidx)
                mult_eng = (
                    nc.gpsimd if gidx in cfg.gp_groups else nc.vector
                )
                ht = htp.tile([128, GFREE], BF16, tag="ht", name="ht")
                mult_eng.tensor_tensor(
                    ht[:, 0:fr], eg[:, 0:fr], cb[:, 0:fr], op=ALU.mult
                )
                ht_by_g[gidx] = (ht, 0)

            def emit_back(gidx):
                joff, ck, gm, ns, gj, ngj, jix = groups[gidx]
                ht, hb = ht_by_g.pop(gidx)
                po = po_by_j[jix]
                kuse = gm - (1 if gj * gm + gm - 1 == cfg.pad_chunk else 0)
                for k in range(kuse):
                    i = gj * gm + k
                    for s in range(ns):
                        nc.tensor.matmul(
                            po[s // 2][:, (s % 2) * D1 : (s % 2 + 1) * D1],
                            ht[:, hb + k * ck + s * 128 :
                               hb + k * ck + (s + 1) * 128],
                            featq[:, i * D1 : (i + 1) * D1],
                            # start zeroes the whole 2KB PSUM bank, so only
                            # the first region of each packed pair sets it
                            start=(gj == 0 and k == 0 and s % 2 == 0),
                            stop=(gj == ngj - 1 and k == kuse - 1),
                            skip_group_check=True,
                        )
                if gj == ngj - 1:  # last group of this j-chunk: drain po,
                    t0 = joff // 128   # divide and ship this slice out now
                    for s in range(ns):
                        nc.vector.tensor_copy(
                            outacc[:, t0 + s, :],
                            po[s // 2][:, (s % 2) * D1 : (s % 2 + 1) * D1],
                        )
                    if cfg.jfin:
                        finalize_j(joff, ns)

            def finalize_j(joff, ns):
                    t0 = joff // 128
                    nc.vector.tensor_scalar(
                        out=dmax[:, t0 : t0 + ns],
                        in0=outacc[:, t0 : t0 + ns, D : D + 1],
                        scalar1=1e-30, scalar2=None, op0=ALU.max,
                    )
                    nc.vector.reciprocal(
                        rden[:, t0 : t0 + ns], dmax[:, t0 : t0 + ns]
                    )
                    for s in range(ns):
                        t = t0 + s
                        nc.vector.tensor_scalar(
                            out=final[:, t * D : (t + 1) * D],
                            in0=outacc[:, t, 0:D],
                            scalar1=rden[:, t : t + 1], scalar2=None,
                            op0=ALU.mult,
                        )
                    nc.sync.dma_start(
                        outd[:, t0 : t0 + ns, :],
                        final[:, t0 * D : (t0 + ns) * D].rearrange(
                            "p (t d) -> p t d", d=D
                        ),
                    )

            if cfg.pair_mult:
                for p in range(0, cfg.ngroups, 2):
                    emit_front_pair(p)
                    emit_back(p)
                    emit_back(p + 1)
            else:
                for g in range(cfg.ngroups + cfg.lag):
                    if g < cfg.ngroups:
                        emit_front(g)
                    if g >= cfg.lag:
                        emit_back(g - cfg.lag)
            if not cfg.jfin:
                for joff, ck, gm, ngj in cfg.jchunks:
                    finalize_j(joff, ck // 128)

    nc.compile()
    return nc


def prepare_inputs(feat, src, dst, beta, cfg):
    feat = np.ascontiguousarray(np.asarray(feat), dtype=np.float32)
    src = np.asarray(src).astype(np.int64)
    dst = np.asarray(dst).astype(np.int64)
    beta = np.asarray(beta, dtype=np.float32).reshape(-1)
    D1 = D + 1

    featp = np.zeros((cfg.npad, D), np.float32)
    featp[: cfg.n_nodes] = feat
    rn = 1.0 / np.maximum(np.linalg.norm(featp, axis=1, keepdims=True), 1e-12)
    normp = featp * rn
    if cfg.fp8_gram:
        # [64, 2, npad]: feature rows split into two 64-row K-tiles
        normT = np.ascontiguousarray(
            normp.T.reshape(2, 64, cfg.npad).transpose(1, 0, 2)
            .astype(ml_dtypes.float8_e4m3fn)
        )
    elif cfg.fp8_plain:
        normT = np.ascontiguousarray(normp.T.astype(ml_dtypes.float8_e4m3fn))
    else:
        normT = np.ascontiguousarray(normp.T.astype(ml_dtypes.bfloat16))

    # featq: [128, mch*(D+1)] bf16; block i col D holds the bias 1.0
    fq = np.ones((128, cfg.mch, D1), dtype=ml_dtypes.bfloat16)
    fq[:, :, :D] = (
        featp.astype(ml_dtypes.bfloat16).reshape(cfg.mch, 128, D).transpose(1, 0, 2)
    )
    fq = np.ascontiguousarray(fq.reshape(128, cfg.mch * D1))

    in_maps = []
    for c in range(cfg.ncores):
        lo = c * cfg.npc
        nmyT = (beta[0] * normp[lo : lo + cfg.npc]).T  # [128, npc]
        if cfg.fp8_gram:
            nmy = np.ascontiguousarray(
                nmyT.reshape(2, 64, cfg.npc).transpose(1, 0, 2)
                .astype(ml_dtypes.float8_e4m3fn)
            )
        elif cfg.fp8_plain:
            nmy = np.ascontiguousarray(nmyT.astype(ml_dtypes.float8_e4m3fn))
        else:
            nmy = np.ascontiguousarray(nmyT.astype(ml_dtypes.bfloat16))
        m = (dst >= lo) & (dst < lo + cfg.npc)
        s_c = src[m]
        d_c = dst[m] - lo
        cnt = np.bincount(
            s_c * cfg.npc + d_c, minlength=cfg.npad * cfg.npc
        ).reshape(cfg.npad, cfg.npc)
        # group-major C layout: per j-chunk, per group: [128, gm*ck]
        blocks = []
        for joff, ck, gm, ngj in cfg.jchunks:
            blk = cnt[:, joff : joff + ck].reshape(ngj, gm, 128, ck)
            blocks.append(blk.transpose(2, 0, 1, 3).reshape(128, ngj * gm * ck))
        ctall = np.concatenate(blocks, axis=1)
        gb = ctall.reshape(128, cfg.ngroups, GFREE)
        if cfg.cast_dma:
            assert ctall.max() <= 255
            ct = np.ascontiguousarray(
                gb[:, cfg.bf_groups, :].reshape(128, -1).astype(np.uint8)
            )
        else:
            ct = np.ascontiguousarray(
                gb[:, cfg.bf_groups, :].reshape(128, -1).astype(ml_dtypes.bfloat16)
            )
        im = {"normT": normT, "normTmy": nmy, "featq": fq, "ct": ct}
        if cfg.u8_groups:
            assert ctall.max() <= 255
            im["ct8"] = np.ascontiguousarray(
                gb[:, cfg.u8_groups, :].reshape(128, -1).astype(np.uint8)
            )
        in_maps.append(im)
    return in_maps


def postprocess(results, cfg):
    parts = []
    for c in range(cfg.ncores):
        o = np.asarray(results[c]["out"], np.float32)  # [128, tt, D]
        parts.append(o.transpose(1, 0, 2).reshape(cfg.npc, D))
    return np.concatenate(parts, axis=0)[: cfg.n_nodes]


_CACHE = {}


def _get_nc(cfg):
    key = (cfg.npad, cfg.ncores, cfg.n_dve_exp, cfg.n_gp_mult, cfg.lag,
           cfg.cb_pair, cfg.jfin, tuple(cfg.u8_groups), cfg.fp8_gram,
           cfg.out_bf16, cfg.fp8_plain, cfg.n_gpx, cfg.pad_chunk,
           cfg.cast_dma, cfg.cbw, cfg.pair_mult)
    if key not in _CACHE:
        _CACHE[key] = build(cfg)
    return _CACHE[key]


def kernel(feat, src, dst, beta):
    cfg = make_cfg()
    nc = _get_nc(cfg)
    in_maps = prepare_inputs(feat, src, dst, beta, cfg)
    res = run_bass_kernel_spmd(nc, in_maps, core_ids=list(range(cfg.ncores)))
    return postprocess(res.results, cfg)

